# revision 1
# baseline (speedup 1.0000x reference)
"""Trainium2 Bass kernel for nn_DenseDSSnetwork (DSS-style GNN, 4 layers).

Math (per layer i, reference semantics):
    h1 = BN_masked(GIN(x, adj, mask; W1,b1,W2,b2,eps))          # big path
    xs = seg_mean(x, graph_idx)                                  # [G,N,D]
    h2 = BN(GIN(xs, original_adj, ones; gsW1..))                 # graph path
    x  = leaky_relu(h1 + h2[graph_idx])
Readout: mean-pool nodes -> seg-mean graphs -> 2-layer MLP -> [G,T].

Device mapping (8 NeuronCores, data parallel over subgraphs; each core owns
8 whole graphs = 512 subgraphs so seg-ops are local; only BN statistics are
all-reduced, one [128,4] f32 allreduce per layer):

  x lives in SBUF feature-major ("fm"): [128 d partitions, 32768 (s,n) cols],
  bf16.  Per layer, chunked by quads (4 subgraph-pairs = 8 subgraphs):
    MM1 (per pair): lhsT = x_fm[:, pair cols]  rhs = W1      -> P node-major
    MM2 (per pair): lhsT = P_pair              rhs = adjaug  -> (t @ W1) fm
        (adjaug = blockdiag(adj_s + (1+eps) I) pre-built on host, so the
         (1+eps)x self-term and the mask are folded into the matmul)
    MM3 (per quad): lhsT = W2                  rhs = lrelu(..+b1) -> v fm
  BN stats come from ACT accum_out on the evacuations plus a strided-AP
  gather of the one deleted-node column per subgraph (mask structure).
  The combine (bn-apply + h2 broadcast-add + leaky) runs on DVE as two
  scalar_tensor_tensor passes, writing x in place.
"""

import os
import numpy as np
import ml_dtypes

import concourse.bass as bass
import concourse.bacc as bacc
import concourse.tile as tile
from concourse import mybir
from concourse.bass_utils import run_bass_kernel_spmd

F32 = mybir.dt.float32
BF16 = mybir.dt.bfloat16
AF = mybir.ActivationFunctionType
ALU = mybir.AluOpType
AX = mybir.AxisListType

G, N, D, L, T = 64, 64, 128, 4, 10
SLOPE = 0.01
BN_EPS = 1e-5
NCORES = 8
GPC = G // NCORES            # graphs per core = 8
SPC = GPC * N                # subgraphs per core = 512
PAIRS = SPC // 2             # 256
QUADS = PAIRS // 4           # 64
OCTS = QUADS // 2            # 32
R = SPC * N                  # fm columns per core = 32768
CNT1 = float(G * N * (N - 1))   # valid rows globally = 4096*63
CNT2 = float(G * N)             # h2 rows globally = 4096

_CACHE = {}
LAST_EXEC_NS = None


def _build_program():
    nc = bacc.Bacc("TRN2", target_bir_lowering=False, debug=False,
                   num_devices=NCORES)

    # ---- DRAM parameters (per core) ----
    xfm_d = nc.dram_tensor("xfm", [D, R], BF16, kind="ExternalInput")
    adj_d = nc.dram_tensor("adjaug", [L, PAIRS, 128, 128], BF16,
                           kind="ExternalInput")
    adj2_d = nc.dram_tensor("adjaug2", [L, GPC, 64, 64], BF16,
                            kind="ExternalInput")
    w1_d = nc.dram_tensor("w1", [L, D, D], BF16, kind="ExternalInput")
    w2_d = nc.dram_tensor("w2", [L, D, D], BF16, kind="ExternalInput")
    gw1_d = nc.dram_tensor("gw1", [L, D, D], BF16, kind="ExternalInput")
    gw2_d = nc.dram_tensor("gw2", [L, D, D], BF16, kind="ExternalInput")
    b1r_d = nc.dram_tensor("b1r", [L, 1, D], BF16, kind="ExternalInput")
    vecs = {}
    for nm in ["b1", "b2", "gb1", "gb2", "bng", "bnb", "bnsg", "bnsb"]:
        vecs[nm] = nc.dram_tensor(nm, [L, D, 1], F32, kind="ExternalInput")
    fw1_d = nc.dram_tensor("fw1", [D, 2 * D], F32, kind="ExternalInput")
    fw2_d = nc.dram_tensor("fw2", [2 * D, T], F32, kind="ExternalInput")
    fb1_d = nc.dram_tensor("fb1", [2 * D, 1], F32, kind="ExternalInput")
    fb2_d = nc.dram_tensor("fb2t", [GPC, T], F32, kind="ExternalInput")
    out_d = nc.dram_tensor("out", [GPC, T], F32, kind="ExternalOutput")

    with tile.TileContext(nc) as tc:
        with (
            tc.tile_pool(name="persist", bufs=1) as pp,
            tc.tile_pool(name="sbP", bufs=3) as sbP,
            tc.tile_pool(name="sbU", bufs=3) as sbU,
            tc.tile_pool(name="sbAdj", bufs=3) as sbAdj,
            tc.tile_pool(name="sbScr", bufs=2) as sbScr,
            tc.tile_pool(name="psQ", bufs=3, space="PSUM") as psQ,
            tc.tile_pool(name="psV", bufs=2, space="PSUM") as psV,
            tc.tile_pool(name="psX", bufs=1, space="PSUM") as psX,
            tc.tile_pool(name="dram", bufs=2, space="DRAM") as dpool,
        ):
            # ---- persistent tensors (per-graph big tiles) ----
            xg = [pp.tile([D, 4096], BF16, name=f"xg{a}", tag=f"xg{a}")
                  for a in range(GPC)]
            hg1 = [pp.tile([D, 4096], BF16, name=f"hg1{a}", tag=f"hg1{a}")
                   for a in range(GPC)]
            h2t = pp.tile([D, SPC], F32, tag="h2t")
            h2np = pp.tile([D, SPC], BF16, tag="h2np")
            h2pu = pp.tile([D, SPC], F32, tag="h2pu")
            wdel = pp.tile([D, SPC], BF16, tag="wdel")
            gdel = pp.tile([D, SPC], BF16, tag="gdel")
            accA = pp.tile([D, OCTS], F32, tag="accA")
            accB = pp.tile([D, OCTS], F32, tag="accB")
            stats = pp.tile([D, 4], F32, tag="stats")
            arres = pp.tile([D, 4], F32, tag="arres")
            zb = pp.tile([D, 1], F32, tag="zb")
            epst = pp.tile([D, 1], F32, tag="epst")
            nc.vector.memset(zb[:], 0.0)
            nc.vector.memset(epst[:], BN_EPS)
            folds = [pp.tile([64, 128], BF16, name=f"fold{a}",
                             tag=f"fold{a}") for a in range(GPC)]
            sc = {nm: pp.tile([D, 1], F32, name="sc_" + nm, tag="sc_" + nm)
                  for nm in ["sumA", "sumB", "dA", "dB", "mean", "ex2", "m2",
                             "var", "sd", "rs", "gk", "cc", "tmp",
                             "mean2", "ex22", "m22", "var2", "sd2", "rs2",
                             "gk2", "cc2", "ccc"]}

            # ---- constants ----
            w1s, w2s, gw1s, gw2s, a2s, b1rs = [], [], [], [], [], []
            vs = {nm: [] for nm in vecs}
            for i in range(L):
                for lst, dt_, nm in ((w1s, w1_d, "w1"), (w2s, w2_d, "w2"),
                                     (gw1s, gw1_d, "gw1"), (gw2s, gw2_d, "gw2")):
                    t = pp.tile([D, D], BF16, name=f"{nm}_{i}", tag=f"{nm}_{i}")
                    nc.sync.dma_start(t[:], dt_.ap()[i])
                    lst.append(t)
                t = pp.tile([1, D], BF16, name=f"b1r_{i}", tag=f"b1r_{i}")
                nc.sync.dma_start(t[:], b1r_d.ap()[i])
                b1rs.append(t)
                row = []
                for a in range(GPC):
                    t = pp.tile([64, 64], BF16, name=f"a2_{i}_{a}",
                                tag=f"a2_{i}_{a}")
                    nc.sync.dma_start(t[:], adj2_d.ap()[i, a])
                    row.append(t)
                a2s.append(row)
                for nm in vecs:
                    t = pp.tile([D, 1], F32, name=f"v_{nm}_{i}",
                                tag=f"v_{nm}_{i}")
                    nc.sync.dma_start(t[:], vecs[nm].ap()[i])
                    vs[nm].append(t)
            ones_r = pp.tile([1, 1024], BF16, tag="ones_r")
            nc.vector.memset(ones_r[:], 1.0)
            fw1 = pp.tile([D, 2 * D], F32, tag="fw1")
            nc.sync.dma_start(fw1[:], fw1_d.ap())
            fw2a = pp.tile([D, T], F32, tag="fw2a")
            fw2b = pp.tile([D, T], F32, tag="fw2b")
            nc.sync.dma_start(fw2a[:], fw2_d.ap()[0:128])
            nc.sync.dma_start(fw2b[:], fw2_d.ap()[128:256])
            fb1a = pp.tile([D, 1], F32, tag="fb1a")
            fb1b = pp.tile([D, 1], F32, tag="fb1b")
            nc.sync.dma_start(fb1a[:], fb1_d.ap()[0:128])
            nc.sync.dma_start(fb1b[:], fb1_d.ap()[128:256])
            fb2t = pp.tile([GPC, T], F32, tag="fb2t")
            nc.sync.dma_start(fb2t[:], fb2_d.ap())

            # ---- x load (fm bf16, host pre-transposed), per graph ----
            for a in range(GPC):
                nc.sync.dma_start(xg[a][:],
                                  xfm_d.ap()[:, a * 4096:(a + 1) * 4096])

            for i in range(L):
                # ================= big path: 32 octs =================
                # per pair: MM1 (x@W1 -> P node-major) and an extra matmul
                # accumulating x@gsW1 into a per-graph PSUM tile (= the
                # seg-sum for the h2 path, free of any tree pass).
                pxs_list = []
                for o in range(OCTS):
                    a = o // 4            # graph of this oct
                    adjq = sbAdj.tile([128, 1024], BF16, tag="adjq")
                    src_q = bass.AP(
                        tensor=adj_d,
                        offset=(i * PAIRS + 8 * o) * 128 * 128,
                        ap=[[128, 128], [128 * 128, 8], [1, 128]])
                    nc.sync.dma_start(
                        adjq[:].rearrange("p (q n) -> p q n", q=8), src_q)
                    if o % 4 == 0:
                        pxs = psX.tile([128, 128], F32, name=f"pxs_{i}_{a}",
                                       tag="pxs")
                        pxs_list.append(pxs)
                    pV = psV.tile([128, 1024], F32, tag="pv")
                    for h in range(2):
                        q = 2 * o + h
                        pP = psQ.tile([128, 512], F32, tag="pq")
                        for k in range(4):
                            pr = 4 * q + k          # global pair
                            lp = pr - a * 32        # pair within graph
                            xsl = xg[a][:, lp * 128:(lp + 1) * 128]
                            nc.tensor.matmul(pP[:, k * 128:(k + 1) * 128],
                                             xsl, w1s[i][:],
                                             start=True, stop=True)
                            nc.tensor.matmul(pxs[:], xsl, gw1s[i][:],
                                             start=(lp == 0),
                                             stop=(lp == 31))
                        P = sbP.tile([128, 512], BF16, tag="psb")
                        nc.vector.tensor_copy(P[:], pP[:])
                        pU = psQ.tile([128, 512], F32, tag="pq")
                        for k in range(4):
                            nc.tensor.matmul(
                                pU[:, k * 128:(k + 1) * 128],
                                P[:, k * 128:(k + 1) * 128],
                                adjq[:, (4 * h + k) * 128:(4 * h + k + 1) * 128],
                                start=True, stop=True)
                        U = sbU.tile([128, 512], BF16, tag="usb")
                        nc.scalar.activation(U[:], pU[:], AF.Lrelu,
                                             bias=vs["b1"][i][:], alpha=SLOPE)
                        nc.tensor.matmul(pV[:, h * 512:(h + 1) * 512],
                                         w2s[i][:], U[:],
                                         start=True, stop=True)
                    lo = (o % 4) * 1024
                    nc.scalar.activation(hg1[a][:, lo:lo + 1024], pV[:],
                                         AF.Identity, bias=vs["b2"][i][:],
                                         accum_out=accA[:, o:o + 1])
                    scr = sbScr.tile([128, 1024], BF16, tag="sqscr")
                    nc.scalar.activation(scr[:], pV[:], AF.Square,
                                         bias=vs["b2"][i][:],
                                         accum_out=accB[:, o:o + 1])
                    if o % 4 == 3:
                        # evac + even/odd fold of this graph's seg-sum
                        t1 = sbScr.tile([128, 128], F32, name=f"t1_{i}_{a}",
                                        tag="t1")
                        nc.scalar.activation(t1[:], pxs[:], AF.Identity,
                                             bias=zb[:])
                        tsh = sbScr.tile([64, 128], F32,
                                         name=f"tsh_{i}_{a}", tag="tsh")
                        nc.sync.dma_start(tsh[:], t1[64:128, :])
                        nc.vector.tensor_add(folds[a][:], t1[0:64, :],
                                             tsh[:])

                # ============== h2 path (tiny, from folds) ==============
                u2 = psQ.tile([128, 512], F32, tag="pq")
                for a in range(GPC):
                    nc.tensor.matmul(u2[:, a * 64:(a + 1) * 64],
                                     folds[a][:], a2s[i][a][:],
                                     start=True, stop=True)
                U2 = sbU.tile([128, 512], BF16, tag="usb")
                nc.scalar.activation(U2[:], u2[:], AF.Lrelu,
                                     bias=vs["gb1"][i][:], alpha=SLOPE)
                v2 = psQ.tile([128, 512], F32, tag="pq")
                nc.tensor.matmul(v2[:], gw2s[i][:], U2[:],
                                 start=True, stop=True)
                nc.scalar.activation(h2t[:], v2[:], AF.Identity,
                                     bias=vs["gb2"][i][:],
                                     accum_out=stats[:, 2:3])
                sq2 = sbScr.tile([128, 512], BF16, tag="dscr")
                nc.scalar.activation(sq2[:], h2t[:], AF.Square, bias=zb[:],
                                     accum_out=stats[:, 3:4])

                # ================= stats + allreduce =================
                nc.vector.tensor_reduce(out=sc["sumA"][:], in_=accA[:],
                                        op=ALU.add, axis=AX.X)
                nc.vector.tensor_reduce(out=sc["sumB"][:], in_=accB[:],
                                        op=ALU.add, axis=AX.X)
                for a in range(GPC):
                    delsrc = bass.AP(tensor=hg1[a].tensor,
                                     offset=hg1[a][:].offset,
                                     ap=[hg1[a][:].ap[0], [65, 64]])
                    nc.vector.tensor_copy(gdel[:, a * 64:(a + 1) * 64],
                                          delsrc)
                dscr = sbScr.tile([128, 512], BF16, tag="dscr")
                nc.scalar.activation(dscr[:], gdel[:], AF.Identity,
                                     bias=zb[:], accum_out=sc["dA"][:])
                dscr2 = sbScr.tile([128, 512], BF16, tag="dscr")
                nc.scalar.activation(dscr2[:], gdel[:], AF.Square,
                                     bias=zb[:], accum_out=sc["dB"][:])
                nc.vector.tensor_tensor(stats[:, 0:1], sc["sumA"][:],
                                        sc["dA"][:], op=ALU.subtract)
                nc.vector.tensor_tensor(stats[:, 1:2], sc["sumB"][:],
                                        sc["dB"][:], op=ALU.subtract)
                ar_in = dpool.tile([D, 4], F32, name=f"ari{i}", tag=f"ari{i}")
                ar_out = dpool.tile([D, 4], F32, name=f"aro{i}",
                                    tag=f"aro{i}")
                nc.gpsimd.dma_start(ar_in[:], stats[:])
                nc.gpsimd.collective_compute(
                    "AllReduce", ALU.add,
                    replica_groups=[list(range(NCORES))],
                    ins=[ar_in.opt()], outs=[ar_out.opt()])
                nc.gpsimd.dma_start(arres[:], ar_out[:])

                # bn parameters (tiny [128,1] ops)
                nc.vector.tensor_scalar_mul(sc["mean"][:], arres[:, 0:1],
                                            1.0 / CNT1)
                nc.vector.tensor_scalar_mul(sc["ex2"][:], arres[:, 1:2],
                                            1.0 / CNT1)
                nc.vector.tensor_mul(sc["m2"][:], sc["mean"][:], sc["mean"][:])
                nc.vector.tensor_tensor(sc["var"][:], sc["ex2"][:],
                                        sc["m2"][:], op=ALU.subtract)
                nc.scalar.activation(sc["sd"][:], sc["var"][:], AF.Sqrt,
                                     bias=epst[:])
                nc.vector.reciprocal(sc["rs"][:], sc["sd"][:])
                nc.vector.tensor_mul(sc["gk"][:], vs["bng"][i][:], sc["rs"][:])
                nc.vector.tensor_mul(sc["tmp"][:], sc["mean"][:], sc["gk"][:])
                nc.vector.tensor_tensor(sc["cc"][:], vs["bnb"][i][:],
                                        sc["tmp"][:], op=ALU.subtract)
                nc.vector.tensor_scalar_mul(sc["mean2"][:], arres[:, 2:3],
                                            1.0 / CNT2)
                nc.vector.tensor_scalar_mul(sc["ex22"][:], arres[:, 3:4],
                                            1.0 / CNT2)
                nc.vector.tensor_mul(sc["m22"][:], sc["mean2"][:],
                                     sc["mean2"][:])
                nc.vector.tensor_tensor(sc["var2"][:], sc["ex22"][:],
                                        sc["m22"][:], op=ALU.subtract)
                nc.scalar.activation(sc["sd2"][:], sc["var2"][:], AF.Sqrt,
                                     bias=epst[:])
                nc.vector.reciprocal(sc["rs2"][:], sc["sd2"][:])
                nc.vector.tensor_mul(sc["gk2"][:], vs["bnsg"][i][:],
                                     sc["rs2"][:])
                nc.vector.tensor_mul(sc["tmp"][:], sc["mean2"][:],
                                     sc["gk2"][:])
                nc.vector.tensor_tensor(sc["cc2"][:], vs["bnsb"][i][:],
                                        sc["tmp"][:], op=ALU.subtract)
                nc.vector.tensor_add(sc["ccc"][:], sc["cc2"][:], sc["cc"][:])
                nc.vector.tensor_scalar(h2np[:], h2t[:], sc["gk2"][:],
                                        sc["ccc"][:], op0=ALU.mult,
                                        op1=ALU.add)
                nc.vector.tensor_scalar(h2pu[:], h2t[:], sc["gk2"][:],
                                        sc["cc2"][:], op0=ALU.mult,
                                        op1=ALU.add)
                nc.vector.scalar_tensor_tensor(wdel[:], h2pu[:], SLOPE,
                                               h2pu[:], op0=ALU.mult,
                                               op1=ALU.max)

                # ====== combine: w=TS(h1*gk) [DVE], z=w+h2x [gpsimd],
                #        x'=Lrelu(z) [ACT], in place per graph ======
                for a in range(GPC):
                    for hh in range(2):
                        lo = hh * 2048
                        h2x = sbScr.tile([128, 2048], BF16, tag="h2x")
                        bsrc = bass.AP(tensor=h2np.tensor,
                                       offset=h2np[:].offset + a * 64,
                                       ap=[h2np[:].ap[0], [0, 32], [1, 64]])
                        nc.sync.dma_start(
                            h2x[:].rearrange("p (s n) -> p s n", s=32), bsrc)
                        wch = sbScr.tile([128, 2048], BF16, tag="wch")
                        nc.vector.tensor_scalar_mul(
                            wch[:], hg1[a][:, lo:lo + 2048], sc["gk"][:])
                        zch = sbScr.tile([128, 2048], BF16, tag="zch")
                        nc.gpsimd.tensor_add(zch[:], wch[:], h2x[:])
                        nc.vector.scalar_tensor_tensor(
                            xg[a][:, lo:lo + 2048], zch[:], SLOPE, zch[:],
                            op0=ALU.mult, op1=ALU.max)
                    deldst = bass.AP(tensor=xg[a].tensor,
                                     offset=xg[a][:].offset,
                                     ap=[xg[a][:].ap[0], [65, 64]])
                    nc.vector.tensor_copy(
                        deldst, wdel[:, a * 64:(a + 1) * 64])

            # ======== readout (DVE tree over n then s, per graph) ========
            tsc = pp.tile([D, 2048], BF16, tag="tsc")
            hg = pp.tile([D, GPC], F32, tag="hg")
            for a in range(GPC):
                v = xg[a][:].rearrange("p (s n) -> p s n", s=64)
                nc.vector.tensor_add(
                    tsc[:, 0:2048].rearrange("p (s n) -> p s n", s=64),
                    v[:, :, 0:32], v[:, :, 32:64])
                w = 16
                while w >= 1:
                    tv = tsc[:, 0:64 * 2 * w].rearrange("p (s n) -> p s n",
                                                        n=2 * w)
                    nc.vector.tensor_add(
                        tsc[:, 0:64 * w].rearrange("p (s n) -> p s n", n=w),
                        tv[:, :, 0:w], tv[:, :, w:2 * w])
                    w //= 2
                nc.vector.tensor_reduce(out=hg[:, a:a + 1],
                                        in_=tsc[:, 0:64],
                                        op=ALU.add, axis=AX.X)
            pF1 = psQ.tile([128, 512], F32, tag="pq")
            pF2 = psQ.tile([128, 512], F32, tag="pq")
            nc.tensor.matmul(pF1[:, 0:GPC], fw1[:, 0:128], hg[:],
                             start=True, stop=True)
            nc.tensor.matmul(pF2[:, 0:GPC], fw1[:, 128:256], hg[:],
                             start=True, stop=True)
            g1a = pp.tile([D, GPC], F32, tag="g1a")
            g1b = pp.tile([D, GPC], F32, tag="g1b")
            nc.scalar.activation(g1a[:], pF1[:, 0:GPC], AF.Lrelu,
                                 bias=fb1a[:], alpha=SLOPE)
            nc.scalar.activation(g1b[:], pF2[:, 0:GPC], AF.Lrelu,
                                 bias=fb1b[:], alpha=SLOPE)
            pO = psQ.tile([GPC, T], F32, tag="pq")
            nc.tensor.matmul(pO[:], g1a[:], fw2a[:], start=True, stop=False)
            nc.tensor.matmul(pO[:], g1b[:], fw2b[:], start=False, stop=True)
            osb = pp.tile([GPC, T], F32, tag="osb")
            nc.scalar.activation(osb[:], pO[:], AF.Identity,
                                 bias=zb[0:GPC, :])
            nc.vector.tensor_add(osb[:], osb[:], fb2t[:])
            nc.sync.dma_start(out_d.ap(), osb[:])

    nc.compile()
    return nc


def _host_prep(inputs):
    """Verify structural assumptions and build per-core input maps."""
    x = np.asarray(inputs["x"], np.float32)            # [S, N, D]
    adj = np.asarray(inputs["adj"], np.float32)        # [S, N, N]
    oadj = np.asarray(inputs["original_adj"], np.float32)
    mask = np.asarray(inputs["mask"])
    omask = np.asarray(inputs["original_mask"])
    gi = np.asarray(inputs["graph_idx"])
    S = G * N
    assert np.array_equal(gi.astype(np.int64),
                          np.repeat(np.arange(G), N)), "graph_idx layout"
    exp_mask = np.ones((S, N), bool)
    exp_mask[np.arange(S), np.arange(S) % N] = False
    assert np.array_equal(mask, exp_mask), "mask structure"
    assert omask.all(), "original_mask must be all ones"

    eps1 = np.asarray(inputs["gnn_eps"], np.float32)
    eps2 = np.asarray(inputs["gs_eps"], np.float32)
    eye = np.eye(N, dtype=np.float32)

    shared = {
        "w1": np.asarray(inputs["gnn_W1"], np.float32).astype(ml_dtypes.bfloat16),
        "w2": np.asarray(inputs["gnn_W2"], np.float32).astype(ml_dtypes.bfloat16),
        "gw1": (np.asarray(inputs["gs_W1"], np.float32) / N).astype(ml_dtypes.bfloat16),
        "gw2": np.asarray(inputs["gs_W2"], np.float32).astype(ml_dtypes.bfloat16),
        "b1": np.asarray(inputs["gnn_b1"], np.float32).reshape(L, D, 1).copy(),
        "b2": np.asarray(inputs["gnn_b2"], np.float32).reshape(L, D, 1).copy(),
        "gb1": np.asarray(inputs["gs_b1"], np.float32).reshape(L, D, 1).copy(),
        "gb2": np.asarray(inputs["gs_b2"], np.float32).reshape(L, D, 1).copy(),
        "bng": np.asarray(inputs["bn_gamma"], np.float32).reshape(L, D, 1).copy(),
        "bnb": np.asarray(inputs["bn_beta"], np.float32).reshape(L, D, 1).copy(),
        "bnsg": np.asarray(inputs["bns_gamma"], np.float32).reshape(L, D, 1).copy(),
        "bnsb": np.asarray(inputs["bns_beta"], np.float32).reshape(L, D, 1).copy(),
        "fw1": (np.asarray(inputs["fin_W1"], np.float32) / (N * (N - 1))).copy(),
        "fw2": np.asarray(inputs["fin_W2"], np.float32).copy(),
        "fb1": np.asarray(inputs["fin_b1"], np.float32).reshape(2 * D, 1).copy(),
        "fb2t": np.tile(np.asarray(inputs["fin_b2"], np.float32),
                        (GPC, 1)).copy(),
        "b1r": np.asarray(inputs["gnn_b1"], np.float32).reshape(
            L, 1, D).astype(ml_dtypes.bfloat16),
    }

    maps = []
    for c in range(NCORES):
        sl = slice(c * SPC, (c + 1) * SPC)
        m = dict(shared)
        m["xfm"] = np.ascontiguousarray(
            x[sl].reshape(SPC * N, D).T).astype(ml_dtypes.bfloat16)
        adjc = adj[sl]                                  # [512, 64, 64]
        bd = np.zeros((L, PAIRS, 128, 128), np.float32)
        for i in range(L):
            bd[i, :, 0:64, 0:64] = adjc[0::2] + (1.0 + eps1[i]) * eye
            bd[i, :, 64:128, 64:128] = adjc[1::2] + (1.0 + eps1[i]) * eye
        m["adjaug"] = bd.astype(ml_dtypes.bfloat16)
        oc = oadj[c * GPC:(c + 1) * GPC]                # [8, 64, 64]
        bd2 = oc[None] + (1.0 + eps2)[:, None, None, None] * eye
        m["adjaug2"] = bd2.astype(ml_dtypes.bfloat16)   # [L, 8, 64, 64]
        maps.append(m)
    return maps


def _install_ntff_hook():
    """The agent image lacks antenv.axon_hooks; synthesize it so trace=True
    can capture NTFF profiles. Only used for profiling runs."""
    import sys
    import types
    try:
        from antenv.axon_hooks import get_axon_ntff_profile_hook  # noqa
        return
    except ImportError:
        pass
    try:
        mod = types.ModuleType("antenv.axon_hooks")
        _hook = [None]
        mod.set_axon_ntff_profile_hook = lambda h: _hook.__setitem__(0, h)
        mod.get_axon_ntff_profile_hook = lambda: _hook[0]
        import antenv
        sys.modules["antenv.axon_hooks"] = mod
        antenv.axon_hooks = mod
        if "/root/.axon_site" not in sys.path:
            sys.path.insert(0, "/root/.axon_site")
        from trn_agent_boot.trn_boot import _ntff_profile_via_ctypes
        mod.set_axon_ntff_profile_hook(
            _ntff_profile_via_ctypes("/opt/axon/libaxon_pjrt.so"))
    except Exception:
        pass


def kernel(**inputs):
    global LAST_EXEC_NS
    if "nc" not in _CACHE:
        _CACHE["nc"] = _build_program()
    nc = _CACHE["nc"]
    maps = _host_prep(inputs)
    trace = bool(int(os.environ.get("KERNEL_TRACE", "0")))
    if trace:
        _install_ntff_hook()
    res = run_bass_kernel_spmd(nc, maps, core_ids=list(range(NCORES)),
                               trace=trace)
    LAST_EXEC_NS = res.exec_time_ns
    out = np.concatenate([res.results[c]["out"] for c in range(NCORES)],
                         axis=0)
    return out.astype(np.float32)

